# revision 6
# baseline (speedup 1.0000x reference)
"""FastRNN Trainium2 kernel v2: h_t = sigmoid(beta)*h_{t-1} + sigmoid(alpha)*tanh(x_t@W + h_{t-1}@U + b).

Strategy (data-parallel over batch, 8 NeuronCores, BC=8 sequences each):
  Everything runs in the TRANSPOSED layout [h-partition, batch] so matmul
  outputs have tiny free dims (cost on the PE scales with output free size)
  and tanh uses all 128 partitions.

  Rescaled state g_t = h_t / sa  (sa=sigmoid(alpha)) gives
      g_t = sb*g_{t-1} + c_t,   c_t = tanh(pre_t),
      pre_{t+1} = wxb_{t+1} + sa*sb*(U^T g_{t-1}) + sa*(U^T c_t)
  so the only work between consecutive tanhs is 16 tiny bf16 matmuls
  (sa*U @ c_t); the g_{t-1} term, the wxb injection, the g-update (one DVE
  op) and output DMA are all off the critical chain.  The host multiplies
  the gathered output by sa during unshard.
"""

from contextlib import ExitStack

import numpy as np

import concourse.bass as bass
import concourse.mybir as mybir
from concourse.bass_utils import run_bass_kernel_spmd
from concourse.tile import TileContext
from concourse.vector_clock import ScopedClock

F32 = mybir.dt.float32
F32R = mybir.dt.float32r
BF16 = mybir.dt.bfloat16
AF = mybir.ActivationFunctionType
ALU = mybir.AluOpType

B, T, I, H = 64, 512, 256, 512
N_CORES = 8
BC = B // N_CORES
KT = H // 128        # 4 h-tiles of 128
KI = I // 128        # 2 input tiles of 128
CH = 32              # timesteps per precompute/output chunk
NCH = T // CH        # number of chunks


class PatchedTileContext(TileContext):
    """The stock tail drain can carry more sem waits than this walrus's
    CTRL-instruction wait slots; spill the excess onto preceding NOPs."""

    def _drain_and_barrier(self, tick_clock, wait_clock):
        nc = self.nc
        drain_inst = nc.sync.drain()
        wait_clock.add_sem_waits(
            drain_inst.ins, ScopedClock({None: tick_clock.global_clock})
        )
        si = drain_inst.ins.sync_info
        waits = list(si.on_wait or []) if si is not None else []
        if len(waits) > 1:
            bb = nc.cur_bb.bb
            idx = bb.instructions.index(drain_inst.ins)
            extra, keep = waits[:-1], waits[-1:]
            si.on_wait = keep
            for i in range(len(extra)):
                nop = nc.sync.nop()
                nsi = nop.ins.sync_info
                if nsi is None:
                    nop.ins.sync_info = mybir.SyncInfo(
                        on_wait=extra[i : i + 1], on_update=[]
                    )
                else:
                    nsi.on_wait = extra[i : i + 1]
                bb.instructions.remove(nop.ins)
                bb.instructions.insert(idx, nop.ins)
                idx += 1
        nc.all_engine_barrier()
        popped = nc._tile_sem_poison_stack.pop()
        assert popped is self._sem_poison
        nc.clear_and_free_semaphores(list(self.sems.allocated().values()))
        nc.all_engine_barrier()


_CTRL_TYPES = ("InstDrain", "InstNop", "InstEventSemOp")


def spill_waits(nc, compute_limit=1, ctrl_limit=1):
    """Move excess per-instruction sync waits onto preceding same-engine NOPs
    (this walrus accepts at most one wait slot per instruction)."""
    for f in nc.m.functions:
        for bb in f.blocks:
            insts = list(bb.instructions)
            for inst in insts:
                si = inst.sync_info
                if si is None or not si.on_wait:
                    continue
                limit = (
                    ctrl_limit
                    if type(inst).__name__ in _CTRL_TYPES
                    else compute_limit
                )
                waits = list(si.on_wait)
                if len(waits) <= limit:
                    continue
                keep = waits[-limit:]
                extra = waits[:-limit]
                si.on_wait = keep
                idx = bb.instructions.index(inst)
                for i in range(0, len(extra), ctrl_limit):
                    nop = nc.engines[inst.engine].nop()
                    nsi = nop.ins.sync_info
                    chunk = extra[i : i + ctrl_limit]
                    if nsi is None:
                        nop.ins.sync_info = mybir.SyncInfo(
                            on_wait=chunk, on_update=[]
                        )
                    else:
                        nsi.on_wait = chunk
                    for f2 in nc.m.functions:
                        for bb2 in f2.blocks:
                            if nop.ins in bb2.instructions:
                                bb2.instructions.remove(nop.ins)
                    bb.instructions.insert(idx, nop.ins)
                    idx += 1


def build_nc(sa: float, sb: float, n_steps: int = T):
    nc = bass.Bass(
        "TRN2", target_bir_lowering=False, debug=False, num_devices=N_CORES
    )
    nch = n_steps // CH

    x = nc.dram_tensor("x", [BC, T, I], F32, kind="ExternalInput")
    up = nc.dram_tensor("up", [H, H], F32, kind="ExternalInput")
    w = nc.dram_tensor("w", [I, H], F32, kind="ExternalInput")
    biastp = nc.dram_tensor("biastp", [128, KT], F32, kind="ExternalInput")
    id128 = nc.dram_tensor("id128", [128, 128], F32, kind="ExternalInput")
    out = nc.dram_tensor(
        "out", [128, KT, n_steps, BC], F32, kind="ExternalOutput"
    )

    with PatchedTileContext(nc) as tc, ExitStack() as ctx:
        pool = lambda **kw: ctx.enter_context(tc.tile_pool(**kw))
        const = pool(name="const", bufs=1)
        w_sb = const.tile([128, KI, H], F32R)
        biastp_sb = const.tile([128, KT], F32)
        id128_sb = const.tile([128, 128], F32R)
        u_sb = const.tile([128, KT, H], F32)
        usa_bf = const.tile([128, KT, H], BF16)   # sa * U
        usb_bf = const.tile([128, KT, H], BF16)   # sa*sb * U
        # wxbT[p, ch, jb, half, b, tsub] = (x@W + bias)[b, ch*32+half*16+tsub,
        #                                               jb*128+p]
        wxbT = const.tile([128, nch, KT, 2, BC, 16], F32R)
        g0 = const.tile([128, KT, BC], F32)

        for k in range(KT):
            nc.gpsimd.dma_start(
                out=u_sb[:, k, :], in_=up[k * 128 : (k + 1) * 128, :]
            )
        for j in range(KI):
            nc.gpsimd.dma_start(
                out=w_sb[:, j, :], in_=w[j * 128 : (j + 1) * 128, :]
            )
        nc.sync.dma_start(out=biastp_sb[:], in_=biastp[:])
        nc.gpsimd.dma_start(out=id128_sb[:], in_=id128[:])

        nc.vector.memset(g0[:], 0.0)
        nc.vector.tensor_scalar_mul(usa_bf[:], u_sb[:], float(sa))
        nc.vector.tensor_scalar_mul(usb_bf[:], u_sb[:], float(sa * sb))

        # ---- wxb precompute machinery (emitted incrementally as closures
        #      so the PE/DVE work spreads between recurrence steps) ----
        xpool = pool(name="xp", bufs=2)
        xtpool = pool(name="xtp", bufs=2)
        tppool = pool(name="tpp", bufs=1, space="PSUM")
        wxpool = pool(name="wxp", bufs=1, space="PSUM")

        def chunk_ops(c):
            """Yield closures; each emits a small batch of instructions for
            precomputing wxbT chunk c (32 timesteps)."""
            t0 = c * CH
            xt = [None, None]
            xT = None
            wps = None

            def load(half):
                xt[half] = xpool.tile([128, I], F32R, tag=f"xt{half}", name=f"xt{half}")
                for b in range(BC):
                    nc.gpsimd.dma_start(
                        out=xt[half][b * 16 : (b + 1) * 16, :],
                        in_=x[b, t0 + half * 16 : t0 + (half + 1) * 16, :],
                    )

            yield lambda: load(0)
            yield lambda: load(1)

            def transpose(half, ki):
                nonlocal xT
                if xT is None:
                    xT = xtpool.tile([128, KI, 2, 128], F32R, name="xT")
                tp = tppool.tile([128, 128], F32R, tag="tp")
                nc.tensor.transpose(
                    tp[:], xt[half][:, ki * 128 : (ki + 1) * 128], id128_sb[:]
                )
                nc.vector.tensor_copy(xT[:, ki, half, :], tp[:])

            for half in range(2):
                for ki in range(KI):
                    yield lambda half=half, ki=ki: transpose(half, ki)

            def mm(jb, ki):
                nonlocal wps
                if wps is None:
                    wps = wxpool.tile([128, KT, 2, BC, 16], F32, tag="wx", name="wps")
                nc.tensor.matmul(
                    wps[:, jb, :, :, :],
                    w_sb[:, ki, jb * 128 : (jb + 1) * 128],
                    xT[:, ki, :, :],
                    start=(ki == 0),
                    stop=(ki == KI - 1),
                )

            for jb in range(KT):
                for ki in range(KI):
                    yield lambda jb=jb, ki=ki: mm(jb, ki)

            def copy(jb):
                nc.vector.tensor_scalar_add(
                    wxbT[:, c, jb, :, :, :],
                    wps[:, jb, :, :, :],
                    biastp_sb[:, jb : jb + 1],
                )

            for jb in range(KT):
                yield lambda jb=jb: copy(jb)

        pending = []

        def pump(k):
            for _ in range(k):
                if not pending:
                    return
                if not pending[0]:
                    pending.pop(0)
                    continue
                op = next(pending[0], None)
                if op is None:
                    pending.pop(0)
                else:
                    op()

        def queue_chunk(c):
            if c < nch:
                pending.append(chunk_ops(c))

        # prime the first two chunks fully, queue the next
        queue_chunk(0)
        queue_chunk(1)
        pump(10_000)
        queue_chunk(2)

        # ---- recurrence ----
        pspool = pool(name="ps", bufs=4, space="PSUM")
        cpool = pool(name="cp", bufs=3)
        gbpool = pool(name="gbp", bufs=3)
        chpool = pool(name="chp", bufs=3)

        def wxslab(t):
            """wxbT slab [128, KT, BC] for step t (1-based)."""
            ti = t - 1
            return wxbT[:, ti // CH, :, (ti % CH) // 16, :, ti % 16]

        def inject(ps, t, stop=False):
            nc.tensor.matmul(
                ps[:], id128_sb[:], wxslab(t), start=True, stop=stop,
            )

        ps = {}
        ps[1] = pspool.tile([128, KT, BC], F32, tag="ps", name="ps1")
        inject(ps[1], 1, stop=True)
        if n_steps >= 2:
            ps[2] = pspool.tile([128, KT, BC], F32, tag="ps", name="ps2")
            inject(ps[2], 2)

        prev_chunk = None
        cur_chunk = None
        gprev = g0[:]          # g_{t-1}
        for t in range(1, n_steps + 1):
            ti = t - 1
            if ti % CH == 0:
                prev_chunk, cur_chunk = cur_chunk, chpool.tile(
                    [128, KT, CH, BC], F32R, tag="chunk", name=f"chunk{ti//CH}"
                )

            # tanh
            cTb = cpool.tile([128, KT, BC], BF16, tag="ctb")
            nc.scalar.activation(cTb[:], ps[t][:], AF.Tanh)

            # chain matmuls: sa*U @ c_t -> ps[t+1]
            if t + 1 <= n_steps:
                for jb in range(KT):
                    for kb in range(KT):
                        nc.tensor.matmul(
                            ps[t + 1][:, jb, :],
                            usa_bf[:, kb, jb * 128 : (jb + 1) * 128],
                            cTb[:, kb, :],
                            start=False,
                            stop=(jb == KT - 1 and kb == KT - 1),
                        )

            # g_t = sb*g_{t-1} + c_t   (also the output buffer)
            gslot = cur_chunk[:, :, ti % CH, :]
            nc.vector.scalar_tensor_tensor(
                out=gslot, in0=gprev, scalar=float(sb), in1=cTb[:],
                op0=ALU.mult, op1=ALU.add,
            )
            gprev = gslot

            if t + 2 <= n_steps:
                gbt = gbpool.tile([128, KT, BC], BF16, tag="gb")
                nc.vector.tensor_copy(gbt[:], gslot)

                ps[t + 2] = pspool.tile([128, KT, BC], F32, tag="ps", name=f"ps{t+2}")
                inject(ps[t + 2], t + 2)
                for jb in range(KT):
                    for kb in range(KT):
                        nc.tensor.matmul(
                            ps[t + 2][:, jb, :],
                            usb_bf[:, kb, jb * 128 : (jb + 1) * 128],
                            gbt[:, kb, :],
                            start=False,
                            stop=False,
                        )

            del ps[t]
            pump(2)
            if ti % CH == CH - 1:
                c = ti // CH
                nc.sync.dma_start(
                    out=out[:, :, c * CH : (c + 1) * CH, :],
                    in_=cur_chunk[:].bitcast(F32),
                )
                queue_chunk(c + 3)

        pump(10_000)

    spill_waits(nc, compute_limit=1)
    return nc


def make_in_maps(x, W, U, bias):
    x = np.ascontiguousarray(np.asarray(x, np.float32))
    W = np.ascontiguousarray(np.asarray(W, np.float32))
    U = np.ascontiguousarray(np.asarray(U, np.float32))
    bias = np.asarray(bias, np.float32)
    biastp = np.ascontiguousarray(
        bias.reshape(KT, 128).T
    )  # [128, KT]: biastp[p, k] = bias[k*128+p]
    id128 = np.eye(128, dtype=np.float32)
    in_maps = []
    for c in range(N_CORES):
        in_maps.append({
            "x": np.ascontiguousarray(x[c * BC : (c + 1) * BC]),
            "up": U, "w": W, "biastp": biastp, "id128": id128,
        })
    return in_maps


_CACHE = {}


def kernel(x, W, U, bias, alpha, beta):
    sa = float(1.0 / (1.0 + np.exp(-np.float64(np.asarray(alpha).reshape(-1)[0]))))
    sb = float(1.0 / (1.0 + np.exp(-np.float64(np.asarray(beta).reshape(-1)[0]))))

    key = (sa, sb)
    if key not in _CACHE:
        _CACHE[key] = build_nc(sa, sb)
    nc = _CACHE[key]

    in_maps = make_in_maps(x, W, U, bias)
    res = run_bass_kernel_spmd(nc, in_maps, list(range(N_CORES))).results

    out = np.empty((B, T, H), np.float32)
    for c in range(N_CORES):
        # out_raw[p, k, t, b] = g_t[b, k*128+p];  h = sa * g
        out[c * BC : (c + 1) * BC] = (
            res[c]["out"].transpose(3, 2, 1, 0).reshape(BC, T, H)
        )
    out *= np.float32(sa)
    return out
